# revision 2
# baseline (speedup 1.0000x reference)
"""Pairwise squared L2 distance (retrieval KNN) on 8 TRN2 NeuronCores.

dist[i, j] = ||x_i||^2 + ||y_j||^2 - 2 * <x_i, y_j>

Sharding: rows of x are split across the 8 cores (data-parallel over n);
y is replicated. Each core computes a [1024, 8192] slab of the distance
matrix.

The kernel is HBM-store-bound (32 MiB of fp32 output per core, ~80 us at
the ~420 GB/s/core measured store rate), so every other engine is kept far
below that floor:

- ONE fp16 matmul for the cross term (the 2e-2 rel-err gate admits plain
  fp16; measured ~2e-4). x is pre-scaled by -2 host-side so the PE
  produces -2*x.y directly.
- The norm terms ride the SAME matmul as 4 augmented contraction rows:
  lhsT aug = [xsq_hi; xsq_lo; 1; 1], rhs aug = [1; 1; ysq_hi; ysq_lo]
  (hi/lo fp16 split keeps the norms exact to ~1e-5). PSUM therefore holds
  the finished distance tile - no arithmetic epilogue at all.
- The mandatory PSUM->SBUF drain is a plain copy, split between ScalarE
  (activation-copy) and VectorE (tensor_copy), each handling half of every
  PSUM group in parallel on disjoint banks.
- Stores stream out per half-group (512 KiB, 4 KiB/partition contiguous).
  Column groups are the outer loop so compute starts after the first
  256 KiB of y has landed.

Inputs are laid out host-side (transpose, fp16 cast, norm rows), so the
device does no transposes and loads only 2.6 MiB.
"""

import numpy as np

import concourse.bass as bass
import concourse.mybir as mybir
import concourse.tile as tile
from concourse import bacc
from concourse.bass import ts
from concourse.bass_utils import run_bass_kernel_spmd

N, M, D = 8192, 8192, 128
NCORES = 8
SLAB = N // NCORES  # 1024 rows of x per core
P = 128  # partitions / m-chunk height
MCH = SLAB // P  # 8 m-chunks per core
NT = 512  # matmul free-dim tile (one fp32 PSUM bank)
GW = 4  # n-chunks per PSUM group (4 banks = 8 KiB/partition)
GCOLS = GW * NT  # 2048
HG = GCOLS // 2  # half-group width (per drain engine / store)
NG = M // GCOLS  # 4 column groups
KAUG = 4  # augmented contraction rows carrying the norm terms

_f32 = mybir.dt.float32
_f16 = mybir.dt.float16

_compiled_nc = None


def _build():
    """Build + compile the single-core Bass program (SPMD across 8 cores)."""
    nc = bacc.Bacc(
        "TRN2",
        target_bir_lowering=False,
        debug=False,
        enable_asserts=False,
        num_devices=NCORES,
    )
    xs2 = nc.dram_tensor("xs2", [D, SLAB], _f16, kind="ExternalInput").ap()
    yh = nc.dram_tensor("yh", [D, M], _f16, kind="ExternalInput").ap()
    au = nc.dram_tensor("au", [KAUG, SLAB], _f16, kind="ExternalInput").ap()
    bu = nc.dram_tensor("bu", [KAUG, M], _f16, kind="ExternalInput").ap()
    dist = nc.dram_tensor("dist", [SLAB, M], _f32, kind="ExternalOutput").ap()

    with tile.TileContext(nc) as tc:
        with (
            tc.tile_pool(name="consts", bufs=1) as cpool,
            tc.tile_pool(name="psum", bufs=2, space="PSUM") as pspool,
            tc.tile_pool(name="osc", bufs=6) as scpool,
            tc.tile_pool(name="ove", bufs=6) as vepool,
        ):
            # First-needed inputs lead so the PE can start ASAP: the first
            # half of y group 0, then x + the aug rows, then the rest of y.
            yh_sb = cpool.tile([D, M], _f16)
            nc.sync.dma_start(yh_sb[:, 0:HG], yh[:, 0:HG])
            xs2_sb = cpool.tile([D, SLAB], _f16)
            nc.sync.dma_start(xs2_sb[:], xs2[:])
            au_sb = cpool.tile([KAUG, SLAB], _f16)
            nc.sync.dma_start(au_sb[:], au[:])
            bu_sb = cpool.tile([KAUG, M], _f16)
            nc.sync.dma_start(bu_sb[:], bu[:])
            nc.sync.dma_start(yh_sb[:, HG:GCOLS], yh[:, HG:GCOLS])
            for g in range(1, NG):
                nc.sync.dma_start(yh_sb[:, ts(g, GCOLS)], yh[:, ts(g, GCOLS)])

            for g in range(NG):
                for mc in range(MCH):
                    ps = pspool.tile([P, GCOLS], _f32, tag="ps")
                    # Main matmuls: weights xs2[mc] held across the group.
                    xw = xs2_sb[:, ts(mc, P)]
                    for jj in range(GW):
                        nc.tensor.matmul(
                            ps[:, ts(jj, NT)],
                            xw,
                            yh_sb[:, g * GCOLS + jj * NT : g * GCOLS + (jj + 1) * NT],
                            start=True,
                            stop=False,
                        )
                    # Aug matmuls fold xsq + ysq into the same PSUM banks.
                    aw = au_sb[:, ts(mc, P)]
                    for jj in range(GW):
                        nc.tensor.matmul(
                            ps[:, ts(jj, NT)],
                            aw,
                            bu_sb[:, g * GCOLS + jj * NT : g * GCOLS + (jj + 1) * NT],
                            start=False,
                            stop=True,
                        )
                    # Drain: ScalarE takes banks 0-1, VectorE banks 2-3.
                    so = scpool.tile([P, HG], _f32, tag="osc")
                    nc.scalar.copy(so[:], ps[:, 0:HG])
                    vo = vepool.tile([P, HG], _f32, tag="ove")
                    nc.vector.tensor_copy(vo[:], ps[:, HG:GCOLS])
                    nc.sync.dma_start(
                        dist[ts(mc, P), g * GCOLS : g * GCOLS + HG], so[:]
                    )
                    nc.sync.dma_start(
                        dist[ts(mc, P), g * GCOLS + HG : (g + 1) * GCOLS], vo[:]
                    )

    nc.compile()
    return nc


def _get_nc():
    global _compiled_nc
    if _compiled_nc is None:
        _compiled_nc = _build()
    return _compiled_nc


def make_in_maps(x: np.ndarray, y: np.ndarray) -> list[dict[str, np.ndarray]]:
    x = np.asarray(x, dtype=np.float32)
    y = np.asarray(y, dtype=np.float32)
    x_sq = np.sum(x * x, axis=1, dtype=np.float32)
    y_sq = np.sum(y * y, axis=1, dtype=np.float32)

    yh = np.ascontiguousarray(y.T.astype(np.float16))  # [D, M]

    ysq_hi = y_sq.astype(np.float16)
    ysq_lo = (y_sq - ysq_hi.astype(np.float32)).astype(np.float16)
    ones_m = np.ones(M, dtype=np.float16)
    bu = np.ascontiguousarray(np.stack([ones_m, ones_m, ysq_hi, ysq_lo]))

    in_maps = []
    for c in range(NCORES):
        sl = slice(c * SLAB, (c + 1) * SLAB)
        xs2 = np.ascontiguousarray((-2.0 * x[sl].T).astype(np.float16))
        xsq = x_sq[sl]
        xsq_hi = xsq.astype(np.float16)
        xsq_lo = (xsq - xsq_hi.astype(np.float32)).astype(np.float16)
        ones_s = np.ones(SLAB, dtype=np.float16)
        au = np.ascontiguousarray(np.stack([xsq_hi, xsq_lo, ones_s, ones_s]))
        in_maps.append({"xs2": xs2, "yh": yh, "au": au, "bu": bu})
    return in_maps


def kernel(x: np.ndarray, y: np.ndarray, **run_kwargs) -> np.ndarray:
    nc = _get_nc()
    in_maps = make_in_maps(x, y)
    res = run_bass_kernel_spmd(nc, in_maps, core_ids=list(range(NCORES)), **run_kwargs)
    out = np.concatenate([res.results[c]["dist"] for c in range(NCORES)], axis=0)
    if run_kwargs:
        kernel.last_results = res
    return out


# revision 4
# speedup vs baseline: 1.4736x; 1.4736x over previous
"""Pairwise squared L2 distance (retrieval KNN) on 8 TRN2 NeuronCores.

dist[i, j] = ||x_i||^2 + ||y_j||^2 - 2 * <x_i, y_j>

Sharding: rows of x are split across the 8 cores (data-parallel over n);
y is replicated. Each core computes a [1024, 8192] slab of the distance
matrix.

Design notes (all engines held at/below the DMA pace):

- ONE fp16 matmul for the cross term (the 2e-2 rel-err gate admits plain
  fp16; measured ~7e-4 end to end). x is pre-scaled by -2 host-side so
  the PE produces -2*x.y directly. Only full-K=128 matmuls are issued:
  small-K matmuls leave most of the PE array idle and the PE_HAM clock
  gate then never releases the 1.2 GHz throttle (measured: a kernel with
  half K=4 matmuls stays cold forever; full-K kernels reach 2.4 GHz).
- Output is stored as fp16 and upcast to fp32 on the host after the
  gather (exact zero-fill upcast; all math happens on-device). This
  halves HBM store traffic - the binding roofline - from 32 MiB to
  16 MiB per core.
- The norm terms are added during the mandatory PSUM->SBUF drain, split
  between the two PSUM-capable engines per 4-bank PSUM group:
  * banks 0-1 -> ScalarE plain activation-copy. Their norms ride a
    full-K=128 zero-padded aug matmul (lhsT rows 0-3 = xsq_hi, xsq_lo,
    1, 1 and zeros below; rhs rows 0-3 = 1, 1, ysq_hi, ysq_lo), so the
    finished value sits in PSUM.
  * banks 2-3 -> VectorE scalar_tensor_tensor: (psum + xsq[p]) + ysq_b,
    with ysq_b a [128, M] partition-broadcast tile built once on the
    otherwise-idle GpSimd engine.
- rhs aug rows 4-127 are don't-care (multiplied by zero weights) but
  must be finite, so they are memset to 0 on VectorE at kernel start.

Inputs are laid out host-side (transpose, fp16 cast, hi/lo norm rows),
so the device does no transposes and loads only ~2.6 MiB.
"""

import numpy as np

import concourse.bass as bass
import concourse.mybir as mybir
import concourse.tile as tile
from concourse import bacc
from concourse.alu_op_type import AluOpType
from concourse.bass import ts
from concourse.bass_utils import run_bass_kernel_spmd

N, M, D = 8192, 8192, 128
NCORES = 8
SLAB = N // NCORES  # 1024 rows of x per core
P = 128  # partitions / m-chunk height
MCH = SLAB // P  # 8 m-chunks per core
NT = 512  # matmul free-dim tile (one fp32 PSUM bank)
GW = 4  # banks per PSUM group (8 KiB/partition)
GCOLS = GW * NT  # 2048
HG = GCOLS // 2  # half-group width (per drain engine / store)
NG = M // GCOLS  # 4 column groups

_f32 = mybir.dt.float32
_f16 = mybir.dt.float16

_compiled_nc = None


def _build():
    """Build + compile the single-core Bass program (SPMD across 8 cores)."""
    nc = bacc.Bacc(
        "TRN2",
        target_bir_lowering=False,
        debug=False,
        enable_asserts=False,
        num_devices=NCORES,
    )
    xs2 = nc.dram_tensor("xs2", [D, SLAB], _f16, kind="ExternalInput").ap()
    yh = nc.dram_tensor("yh", [D, M], _f16, kind="ExternalInput").ap()
    agw = nc.dram_tensor("agw", [D, SLAB], _f16, kind="ExternalInput").ap()
    bgs = nc.dram_tensor("bgs", [4, M], _f16, kind="ExternalInput").ap()
    xsq = nc.dram_tensor("xsq", [P, MCH], _f32, kind="ExternalInput").ap()
    ysq = nc.dram_tensor("ysq", [1, M], _f32, kind="ExternalInput").ap()
    dist16 = nc.dram_tensor("dist16", [SLAB, M], _f16, kind="ExternalOutput").ap()

    with tile.TileContext(nc) as tc:
        with (
            tc.tile_pool(name="consts", bufs=1) as cpool,
            tc.tile_pool(name="psum", bufs=2, space="PSUM") as pspool,
            tc.tile_pool(name="osc", bufs=6) as scpool,
            tc.tile_pool(name="ove", bufs=6) as vepool,
        ):
            # Aug rhs: rows 0-3 from HBM, rows 4-127 zeroed on-chip (the
            # zero lhsT rows multiply them, so they only must be finite).
            # Partition slices must start aligned, so memset all 128 rows
            # first; the row 0-3 DMA below overwrites the top.
            bu_sb = cpool.tile([D, M], _f16)
            for g in range(NG):
                nc.vector.memset(bu_sb[:, ts(g, GCOLS)], 0.0)

            # Small-first load order so the ramp is short.
            ysq_row = cpool.tile([1, M], _f32)
            nc.sync.dma_start(ysq_row[:], ysq[:])
            xsq_sb = cpool.tile([P, MCH], _f32)
            nc.sync.dma_start(xsq_sb[:], xsq[:])
            yh_sb = cpool.tile([D, M], _f16)
            nc.sync.dma_start(yh_sb[:, 0:HG], yh[:, 0:HG])
            xs2_sb = cpool.tile([D, SLAB], _f16)
            nc.sync.dma_start(xs2_sb[:], xs2[:])
            agw_sb = cpool.tile([D, SLAB], _f16)
            nc.sync.dma_start(agw_sb[:], agw[:])
            nc.sync.dma_start(bu_sb[0:4, :], bgs[:])
            nc.sync.dma_start(yh_sb[:, HG:GCOLS], yh[:, HG:GCOLS])
            for g in range(1, NG):
                nc.sync.dma_start(yh_sb[:, ts(g, GCOLS)], yh[:, ts(g, GCOLS)])

            # ysq_b[p, j] = y_sq[j] (exact fp32) for the VectorE drain.
            ysq_b = cpool.tile([P, M], _f32)
            for g in range(NG):
                nc.gpsimd.partition_broadcast(
                    ysq_b[:, ts(g, GCOLS)], ysq_row[0:1, ts(g, GCOLS)]
                )

            for g in range(NG):
                for mc in range(MCH):
                    ps = pspool.tile([P, GCOLS], _f32, tag="ps")
                    c0 = g * GCOLS
                    # Main matmuls: weights xs2[mc] held across the group.
                    # Banks 2-3 are complete after this; banks 0-1 wait
                    # for the aug pass.
                    xw = xs2_sb[:, ts(mc, P)]
                    for jj in range(GW):
                        nc.tensor.matmul(
                            ps[:, ts(jj, NT)],
                            xw,
                            yh_sb[:, c0 + jj * NT : c0 + (jj + 1) * NT],
                            start=True,
                            stop=(jj >= 2),
                        )
                    # Full-K aug matmuls fold xsq + ysq into banks 0-1.
                    aw = agw_sb[:, ts(mc, P)]
                    for jj in range(2):
                        nc.tensor.matmul(
                            ps[:, ts(jj, NT)],
                            aw,
                            bu_sb[:, c0 + jj * NT : c0 + (jj + 1) * NT],
                            start=False,
                            stop=True,
                        )
                    # Drain to fp16: ScalarE copies banks 0-1 (finished in
                    # PSUM); VectorE adds both norms to banks 2-3.
                    so = scpool.tile([P, HG], _f16, tag="osc")
                    nc.scalar.copy(so[:], ps[:, 0:HG])
                    vo = vepool.tile([P, HG], _f16, tag="ove")
                    nc.vector.scalar_tensor_tensor(
                        vo[:],
                        ps[:, HG:GCOLS],
                        xsq_sb[:, mc : mc + 1],
                        ysq_b[:, c0 + HG : c0 + GCOLS],
                        AluOpType.add,
                        AluOpType.add,
                    )
                    nc.sync.dma_start(dist16[ts(mc, P), c0 : c0 + HG], so[:])
                    nc.sync.dma_start(
                        dist16[ts(mc, P), c0 + HG : c0 + GCOLS], vo[:]
                    )

    nc.compile()
    return nc


def _get_nc():
    global _compiled_nc
    if _compiled_nc is None:
        _compiled_nc = _build()
    return _compiled_nc


def make_in_maps(x: np.ndarray, y: np.ndarray) -> list[dict[str, np.ndarray]]:
    x = np.asarray(x, dtype=np.float32)
    y = np.asarray(y, dtype=np.float32)
    x_sq = np.sum(x * x, axis=1, dtype=np.float32)
    y_sq = np.sum(y * y, axis=1, dtype=np.float32)

    yh = np.ascontiguousarray(y.T.astype(np.float16))  # [D, M]

    ysq_hi = y_sq.astype(np.float16)
    ysq_lo = (y_sq - ysq_hi.astype(np.float32)).astype(np.float16)
    ones_m = np.ones(M, dtype=np.float16)
    bgs = np.ascontiguousarray(np.stack([ones_m, ones_m, ysq_hi, ysq_lo]))
    ysq_in = np.ascontiguousarray(y_sq.reshape(1, M))

    in_maps = []
    for c in range(NCORES):
        sl = slice(c * SLAB, (c + 1) * SLAB)
        xs2 = np.ascontiguousarray((-2.0 * x[sl].T).astype(np.float16))
        xsq = x_sq[sl]
        xsq_hi = xsq.astype(np.float16)
        xsq_lo = (xsq - xsq_hi.astype(np.float32)).astype(np.float16)
        agw = np.zeros((D, SLAB), dtype=np.float16)
        agw[0] = xsq_hi
        agw[1] = xsq_lo
        agw[2] = 1.0
        agw[3] = 1.0
        # [P, MCH]: column mc holds x_sq for rows mc*128..mc*128+127
        xsq_in = np.ascontiguousarray(xsq.reshape(MCH, P).T)
        in_maps.append(
            {
                "xs2": xs2,
                "yh": yh,
                "agw": agw,
                "bgs": bgs,
                "xsq": xsq_in,
                "ysq": ysq_in,
            }
        )
    return in_maps


def kernel(x: np.ndarray, y: np.ndarray, **run_kwargs) -> np.ndarray:
    nc = _get_nc()
    in_maps = make_in_maps(x, y)
    res = run_bass_kernel_spmd(nc, in_maps, core_ids=list(range(NCORES)), **run_kwargs)
    out = np.concatenate(
        [res.results[c]["dist16"] for c in range(NCORES)], axis=0
    ).astype(np.float32)
    if run_kwargs:
        kernel.last_results = res
    return out
